# revision 8
# baseline (speedup 1.0000x reference)
"""CBAM block kernel for Trainium2, 8-core data-parallel.

Computation (per image, C=256 channels, HW=56*56=3136 pixels):
  channel attention: spatial avg/max pool -> tiny MLP (BN+tanh) -> sigmoid -> ca[C]
  spatial attention: channel mean/max of ca*x -> reflect-pad 3x3 conv (2->1 ch)
                     -> two folded BNs -> sigmoid -> sa[HW]
  out = relu(fbn_scale * (x*ca*sa + x) + fbn_bias)
      = relu(x (*) M + bfb),  M = (sf*ca) (x) sa + sf (x) 1   (rank-2, via PE)

Sharding: pure data parallel, 4 images per NeuronCore, params replicated.

Key device techniques:
  - spatial sum/max pools: DVE tensor_scalar accum_out (op1=add / op1=max) at 2x
    rate, fused with the f32->bf16 cast of x
  - MLP runs channels-on-partitions so BN scale/bias are per-partition ACT args
  - channel max of ca*x: PE matmuls  x_bf16_block^T @ diag(ca_bf16)  transpose
    112-pixel blocks into PSUM supertiles, DVE reduce_max over the channel
    (free) axis; diag(ca) built by gpsimd affine_select
  - channel sum of ca*x: PE matmul with lhsT=ca (float32r fast path)
  - 3x3 conv: 3 PE matmuls against host-im2col'd banded weight matrices
    (both BNs + conv bias + 1/C mean factor folded in on host)
  - final: M from a K=2 outer-product matmul (float32r), DVE hadamard, relu
    with per-channel bias on ACT/DVE
"""

import os
from contextlib import ExitStack

import numpy as np

import concourse.bacc as bacc
import concourse.bass as bass
import concourse.mybir as mybir
import concourse.tile as tile
from concourse import bass_utils

F32 = mybir.dt.float32
F32R = mybir.dt.float32r
BF16 = mybir.dt.bfloat16
Alu = mybir.AluOpType
Act = mybir.ActivationFunctionType
AxX = mybir.AxisListType.X

B, C, H, W = 32, 256, 56, 56
HW = H * W                      # 3136
NCORES = 8
BLOC = B // NCORES              # 4 images per core
NCH = 2                         # channel chunks of 128
MID = C // 16                   # 16
GP = 112                        # pixels per transpose block (2 rows)
NGRP = HW // GP                 # 28
SUPG = 4                        # groups per psum supertile
NSUP = NGRP // SUPG             # 7
PIECE = 392                     # free-dim piece for the M/hadamard stage
NPIECE = HW // PIECE            # 8
OUTW = 1568                     # batched output store width (4 pieces)
APIECE = 448                    # avg-path psum piece
NAPIECE = HW // APIECE          # 7


# ---------------------------------------------------------------------------
# device program
# ---------------------------------------------------------------------------

def _build_program():
    nc = bacc.Bacc(
        "TRN2",
        target_bir_lowering=False,
        debug=False,
        enable_asserts=False,
        num_devices=NCORES,
    )

    x_d = nc.dram_tensor("x_shard", [BLOC, C, HW], F32, kind="ExternalInput").ap()
    y_d = nc.dram_tensor("y_shard", [BLOC, C, HW], F32, kind="ExternalOutput").ap()
    w1a_d = nc.dram_tensor("w1t_avg", [C, MID], F32, kind="ExternalInput").ap()
    w1m_d = nc.dram_tensor("w1t_max", [C, MID], F32, kind="ExternalInput").ap()
    w2t_d = nc.dram_tensor("w2t", [MID, C], F32, kind="ExternalInput").ap()
    mlpv_d = nc.dram_tensor("mlp_vec", [MID, 2], F32, kind="ExternalInput").ap()
    chv_d = nc.dram_tensor("ch_vec", [C, 4], F32, kind="ExternalInput").ap()
    sfrow_d = nc.dram_tensor("sf_rows", [NCH, 128], BF16, kind="ExternalInput").ap()
    bmat_d = nc.dram_tensor("bmat", [116, 168], BF16, kind="ExternalInput").ap()
    ones_d = nc.dram_tensor("ones_row", [1, HW], BF16, kind="ExternalInput").ap()
    cst_d = nc.dram_tensor("conv_cst", [1, 1], F32, kind="ExternalInput").ap()

    with tile.TileContext(nc) as tc:
        with ExitStack() as ctx:
            _trace_kernel(ctx, tc, y_d, x_d, w1a_d, w1m_d, w2t_d, mlpv_d, chv_d,
                          sfrow_d, bmat_d, ones_d, cst_d)
    nc.compile()
    return nc


def _trace_kernel(ctx, tc, y_d, x_d, w1a_d, w1m_d, w2t_d, mlpv_d, chv_d,
                  sfrow_d, bmat_d, ones_d, cst_d):
    nc = tc.nc

    consts = ctx.enter_context(tc.tile_pool(name="consts", bufs=1))
    px = ctx.enter_context(tc.tile_pool(name="px", bufs=2 * BLOC))
    pxb = ctx.enter_context(tc.tile_pool(name="pxb", bufs=4))
    pstat = ctx.enter_context(tc.tile_pool(name="pstat", bufs=12))
    pdiag = ctx.enter_context(tc.tile_pool(name="pdiag", bufs=4))
    prow = ctx.enter_context(tc.tile_pool(name="prow", bufs=2))
    pout = ctx.enter_context(tc.tile_pool(name="pout", bufs=4))
    ps_xt = ctx.enter_context(tc.tile_pool(name="ps_xt", bufs=2, space="PSUM"))
    ps_m = ctx.enter_context(tc.tile_pool(name="ps_m", bufs=2, space="PSUM"))
    ps_sm = ctx.enter_context(tc.tile_pool(name="ps_sm", bufs=2, space="PSUM"))

    # ---- constants into SBUF ----
    w1a = [consts.tile([128, MID], F32, tag=f"w1a{c}", name=f"w1a{c}") for c in range(NCH)]
    w1m = [consts.tile([128, MID], F32, tag=f"w1m{c}", name=f"w1m{c}") for c in range(NCH)]
    for c in range(NCH):
        nc.sync.dma_start(out=w1a[c], in_=w1a_d[c * 128:(c + 1) * 128, :])
        nc.sync.dma_start(out=w1m[c], in_=w1m_d[c * 128:(c + 1) * 128, :])
    w2t = consts.tile([MID, C], F32, tag="w2t")
    nc.sync.dma_start(out=w2t, in_=w2t_d)
    mlpv = consts.tile([MID, 2], F32, tag="mlpv")
    nc.sync.dma_start(out=mlpv, in_=mlpv_d)
    chv = [consts.tile([128, 4], F32, tag=f"chv{c}", name=f"chv{c}") for c in range(NCH)]
    for c in range(NCH):
        nc.sync.dma_start(out=chv[c], in_=chv_d[c * 128:(c + 1) * 128, :])
    bmat = consts.tile([116, 168], BF16, tag="bmat")
    nc.sync.dma_start(out=bmat, in_=bmat_d)
    cst56 = consts.tile([56, 1], F32, tag="cst56")
    nc.sync.dma_start(out=cst56, in_=cst_d.to_broadcast((56, 1)))

    ones1 = consts.tile([128, 1], F32, tag="ones1")
    nc.vector.memset(ones1, 1.0)
    ident = consts.tile([128, 128], F32, tag="ident")
    nc.gpsimd.affine_select(
        out=ident, in_=ones1.broadcast_to((128, 128)), pattern=[[-1, 128]],
        compare_op=Alu.is_equal, fill=0.0, base=0, channel_multiplier=1,
    )

    for i in range(BLOC):
        # ---- load + pools + bf16 cast ----
        xc, xb, s_sum, s_max = [], [], [], []
        for c in range(NCH):
            xt = px.tile([128, HW], F32, tag="x")
            nc.sync.dma_start(out=xt, in_=x_d[i, c * 128:(c + 1) * 128, :])
            xc.append(xt)
            xbt = pxb.tile([128, HW], BF16, tag="xb")
            ss = pstat.tile([128, 1], F32, tag="ssum")
            sm = pstat.tile([128, 1], F32, tag="smax")
            nc.vector.tensor_scalar(out=xbt, in0=xt, scalar1=1.0, scalar2=None,
                                    op0=Alu.mult, op1=Alu.add, accum_out=ss)
            xtrash = pxb.tile([128, HW], BF16, tag="xbtrash", bufs=1)
            nc.vector.tensor_scalar(out=xtrash, in0=xbt, scalar1=1.0,
                                    scalar2=None, op0=Alu.mult, op1=Alu.max,
                                    accum_out=sm)
            xb.append(xbt)
            s_sum.append(ss)
            s_max.append(sm)

        # ---- channel-attention MLP (channels on partitions) ----
        havg = ps_sm.tile([MID, 1], F32, tag="sm")
        hmax = ps_sm.tile([MID, 1], F32, tag="sm")
        for c in range(NCH):
            nc.tensor.matmul(out=havg, lhsT=w1a[c], rhs=s_sum[c],
                             start=(c == 0), stop=(c == 1))
        for c in range(NCH):
            nc.tensor.matmul(out=hmax, lhsT=w1m[c], rhs=s_max[c],
                             start=(c == 0), stop=(c == 1))
        tha = pstat.tile([MID, 1], F32, tag="tha")
        thm = pstat.tile([MID, 1], F32, tag="thm")
        nc.scalar.activation(out=tha, in_=havg, func=Act.Tanh,
                             bias=mlpv[:, 1:2], scale=mlpv[:, 0:1])
        nc.scalar.activation(out=thm, in_=hmax, func=Act.Tanh,
                             bias=mlpv[:, 1:2], scale=mlpv[:, 0:1])

        ca, ca_bf, diag, lhsT2 = [], [], [], []
        for c in range(NCH):
            oa = ps_sm.tile([128, 1], F32, tag="sm")
            om = ps_sm.tile([128, 1], F32, tag="sm")
            nc.tensor.matmul(out=oa, lhsT=w2t[:, c * 128:(c + 1) * 128], rhs=tha,
                             start=True, stop=True)
            nc.tensor.matmul(out=om, lhsT=w2t[:, c * 128:(c + 1) * 128], rhs=thm,
                             start=True, stop=True)
            ta = pstat.tile([128, 1], F32, tag="ta")
            tm = pstat.tile([128, 1], F32, tag="tm")
            nc.scalar.activation(out=ta, in_=oa, func=Act.Tanh,
                                 bias=chv[c][:, 1:2], scale=chv[c][:, 0:1])
            nc.scalar.activation(out=tm, in_=om, func=Act.Tanh,
                                 bias=chv[c][:, 1:2], scale=chv[c][:, 0:1])
            sab = pstat.tile([128, 1], F32, tag="sab")
            nc.vector.tensor_tensor(out=sab, in0=ta, in1=tm, op=Alu.add)
            cat = pstat.tile([128, 1], F32, tag="ca")
            nc.scalar.activation(out=cat, in_=sab, func=Act.Sigmoid)
            ca.append(cat)

            dg = pdiag.tile([128, 128], BF16, tag="diag")
            nc.gpsimd.affine_select(
                out=dg, in_=cat.broadcast_to((128, 128)), pattern=[[-1, 128]],
                compare_op=Alu.is_equal, fill=0.0, base=0, channel_multiplier=1,
            )
            diag.append(dg)

            cab = pstat.tile([128, 1], BF16, tag="cab")
            nc.vector.tensor_copy(out=cab, in_=cat)
            ca_bf.append(cab)
            sfca = pstat.tile([128, 1], F32, tag="sfca")
            nc.vector.tensor_scalar(out=sfca, in0=cat, scalar1=chv[c][:, 2:3],
                                    scalar2=None, op0=Alu.mult)
            sfcaT = ps_sm.tile([1, 128], F32, tag="sm")
            nc.tensor.transpose(out=sfcaT, in_=sfca, identity=ident)
            l2 = prow.tile([2, 128], BF16, tag="lhsT2", bufs=4)
            nc.vector.tensor_copy(out=l2[0:1, :], in_=sfcaT)
            nc.scalar.dma_start(out=l2[1:2, :], in_=sfrow_d[c:c + 1, :])
            lhsT2.append(l2)

        # ---- channel max of ca*x (transpose blocks via diag matmul) ----
        mx = prow.tile([GP, NGRP], F32, tag="mx")
        for s in range(NSUP):
            sup = ps_xt.tile([GP, SUPG, C], F32, tag="sup")
            for gg in range(SUPG):
                g = s * SUPG + gg
                for c in range(NCH):
                    nc.tensor.matmul(
                        out=sup[:, gg, c * 128:(c + 1) * 128],
                        lhsT=xb[c][:, g * GP:(g + 1) * GP], rhs=diag[c],
                        start=True, stop=True,
                    )
            nc.vector.tensor_reduce(out=mx[:, s * SUPG:(s + 1) * SUPG], in_=sup,
                                    axis=AxX, op=Alu.max)
        mxT_p = ps_sm.tile([NGRP, GP], F32, tag="sm")
        nc.tensor.transpose(out=mxT_p, in_=mx, identity=ident[0:GP, 0:GP])
        mxT = prow.tile([NGRP, GP], BF16, tag="mxT")
        nc.vector.tensor_copy(out=mxT, in_=mxT_p)

        # ---- channel sum of ca*x (float32r matmul with ca weights) ----
        avg_row = prow.tile([1, HW], BF16, tag="avg_row")
        for p in range(NAPIECE):
            sl = slice(p * APIECE, (p + 1) * APIECE)
            ap = ps_sm.tile([1, APIECE], F32, tag="sm")
            for c in range(NCH):
                nc.tensor.matmul(out=ap, lhsT=ca_bf[c], rhs=xb[c][:, sl],
                                 start=(c == 0), stop=(c == 1))
            nc.scalar.activation(out=avg_row[0:1, sl], in_=ap, func=Act.Copy)

        # ---- assemble reflect-padded conv input S = [avg(58) ; max(58)] ----
        S = prow.tile([116, 58], BF16, tag="S")
        nc.scalar.dma_start(out=S[1:57, 1:57], in_=avg_row)
        nc.scalar.dma_start(out=S[59:115, 1:57], in_=mxT)
        nc.scalar.dma_start(out=S[0:1, 1:57], in_=S[2:3, 1:57])
        nc.scalar.dma_start(out=S[57:58, 1:57], in_=S[55:56, 1:57])
        nc.scalar.dma_start(out=S[58:59, 1:57], in_=S[60:61, 1:57])
        nc.scalar.dma_start(out=S[115:116, 1:57], in_=S[113:114, 1:57])
        nc.scalar.dma_start(out=S[:, 0:1], in_=S[:, 2:3])
        nc.scalar.dma_start(out=S[:, 57:58], in_=S[:, 55:56])

        # ---- 3x3 conv as 3 banded matmuls + sigmoid -> sa ----
        conv = ps_sm.tile([56, 56], F32, tag="sm")
        for dx in range(3):
            nc.tensor.matmul(out=conv, lhsT=bmat[:, dx * 56:(dx + 1) * 56],
                             rhs=S[:, dx:dx + 56],
                             start=(dx == 0), stop=(dx == 2))
        sa56 = prow.tile([56, 56], BF16, tag="sa56")
        nc.scalar.activation(out=sa56, in_=conv, func=Act.Sigmoid,
                             bias=cst56, scale=1.0)

        rhs2 = prow.tile([2, HW], BF16, tag="rhs2")
        nc.scalar.dma_start(out=rhs2[0:1, :], in_=sa56)
        nc.scalar.dma_start(out=rhs2[1:2, :], in_=ones_d)

        # ---- final: M = sfca (x) sa + sf (x) 1 ; out = relu(x*M + bfb) ----
        for c in range(NCH):
            for half in range(HW // OUTW):
                ot = pout.tile([128, OUTW], F32, tag="ot", bufs=3)
                for pp in range(OUTW // PIECE):
                    base = half * OUTW + pp * PIECE
                    sl = slice(base, base + PIECE)
                    osl = slice(pp * PIECE, (pp + 1) * PIECE)
                    mp = ps_m.tile([128, PIECE], F32, tag="mp")
                    nc.tensor.matmul(out=mp, lhsT=lhsT2[c], rhs=rhs2[:, sl],
                                     start=True, stop=True)
                    nc.vector.tensor_tensor(out=ot[:, osl], in0=xc[c][:, sl],
                                            in1=mp, op=Alu.mult)
                    nc.scalar.activation(out=ot[:, osl], in_=ot[:, osl],
                                         func=Act.Relu,
                                         bias=chv[c][:, 3:4], scale=1.0)
                nc.sync.dma_start(
                    out=y_d[i, c * 128:(c + 1) * 128,
                            half * OUTW:(half + 1) * OUTW], in_=ot)


# ---------------------------------------------------------------------------
# host-side parameter folding
# ---------------------------------------------------------------------------

def _fold_params(inp):
    f = lambda a: np.asarray(a, dtype=np.float32)

    s1 = f(inp["bn1_g"]) / np.sqrt(f(inp["bn1_v"]) + 1e-5)
    b1 = f(inp["bn1_b"]) - f(inp["bn1_m"]) * s1
    s2 = f(inp["bn2_g"]) / np.sqrt(f(inp["bn2_v"]) + 1e-5)
    b2 = f(inp["bn2_b"]) - f(inp["bn2_m"]) * s2
    sf = f(inp["fbn_g"]) / np.sqrt(f(inp["fbn_v"]) + 1e-5)
    bfb = f(inp["fbn_b"]) - f(inp["fbn_m"]) * sf

    w1 = f(inp["w1"])                      # [MID, C]
    w2 = f(inp["w2"])                      # [C, MID]
    w1t_avg = np.ascontiguousarray((w1 / HW).T)     # [C, MID]
    w1t_max = np.ascontiguousarray(w1.T)            # [C, MID]
    w2t = np.ascontiguousarray(w2.T)                # [MID, C]
    mlp_vec = np.stack([s1, b1], axis=1)            # [MID, 2]
    ch_vec = np.stack([s2, b2, sf, bfb], axis=1)    # [C, 4]
    import ml_dtypes
    sf_rows = np.ascontiguousarray(sf.reshape(NCH, 128).astype(ml_dtypes.bfloat16))

    # spatial conv folding: two BNs + conv bias + channel-mean divisor
    a1 = f(inp["sbn1_g"])[0] / np.sqrt(f(inp["sbn1_v"])[0] + 1e-3)
    c1 = f(inp["sbn1_b"])[0] - f(inp["sbn1_m"])[0] * a1
    a2 = f(inp["sbn2_g"])[0] / np.sqrt(f(inp["sbn2_v"])[0] + 1e-5)
    c2 = f(inp["sbn2_b"])[0] - f(inp["sbn2_m"])[0] * a2
    amul = a1 * a2
    cst = a2 * (a1 * f(inp["sconv_b"])[0] + c1) + c2

    wsp = f(inp["sconv_w"])[0]             # [2, 3, 3]
    w_eff = np.stack([wsp[0] * amul / C, wsp[1] * amul])  # [2(ic), 3(dy), 3(dx)]

    bmat = np.zeros((116, 168), np.float32)   # [ic*58+yp, dx*56+y]
    for ic in range(2):
        for dx in range(3):
            for y in range(56):
                for dy in range(3):
                    bmat[ic * 58 + y + dy, dx * 56 + y] = w_eff[ic, dy, dx]

    return {
        "w1t_avg": w1t_avg, "w1t_max": w1t_max, "w2t": w2t,
        "mlp_vec": np.ascontiguousarray(mlp_vec),
        "ch_vec": np.ascontiguousarray(ch_vec),
        "sf_rows": sf_rows, "bmat": bmat.astype(ml_dtypes.bfloat16),
        "ones_row": np.ones((1, HW), __import__("ml_dtypes").bfloat16),
        "conv_cst": np.full((1, 1), cst, np.float32),
    }


_NC_CACHE = {}


def _get_program():
    if "nc" not in _NC_CACHE:
        _NC_CACHE["nc"] = _build_program()
    return _NC_CACHE["nc"]


def kernel(**inputs) -> np.ndarray:
    nc = _get_program()
    params = _fold_params(inputs)
    x = np.asarray(inputs["x"], dtype=np.float32).reshape(B, C, HW)

    in_maps = []
    for core in range(NCORES):
        shard = np.ascontiguousarray(x[core * BLOC:(core + 1) * BLOC])
        in_maps.append({"x_shard": shard, **params})

    res = bass_utils.run_bass_kernel_spmd(nc, in_maps, core_ids=list(range(NCORES)))
    out = np.concatenate([r["y_shard"] for r in res.results], axis=0)
    return out.reshape(B, C, H, W).astype(np.float32)


if __name__ == "__main__":
    rng = np.random.default_rng(0)
    demo = {"x": rng.standard_normal((B, C, H, W), dtype=np.float32)}
    raise SystemExit("run test.py instead")
